# revision 49
# baseline (speedup 1.0000x reference)
"""Trainium2 Bass kernel for nn_Encoder_dpm (GNN message-passing encoder).

Contract: kernel(**inputs) takes the FULL unsharded inputs (as produced by
setup_inputs) and returns the FULL output tuple (h0 [N,128], v0 [N,128,3],
ea [E,128]) as float32 numpy arrays.

Sharding strategy (destination-node sharding -> NO collectives needed):
  - The 25000 nodes are split into 8 contiguous ranges of 3125 nodes.
  - Core c processes exactly the edges whose *destination* j falls in its
    range, so the segment-sums (agg and v0) are entirely local to the core.
  - Node features f = species @ Wa + ba are computed (redundantly) on every
    core and stored as a row table in HBM so each core can gather f[i] for
    arbitrary source nodes i.
  - Per-core, local nodes are bin-packed into V "virtual blocks" of 128 node
    slots such that each block owns ~the same number of incoming edges.  The
    edge list is grouped by block and padded to a fixed B edges per block,
    giving every core an identical static program.
  - The segment-sum is computed per block as a one-hot matmul on the tensor
    engine, accumulating [128 nodes x (agg | v0x | v0y | v0z)] = [128, 512]
    in a PSUM bank across the block's edges.

Per-edge pipeline (all matmuls on the PE with N=512 free dim where possible):
  ea   = MLP_b(edge_attr)                  (features-on-partition layout)
  h_s  = Ws1a.T@fi_T + Ws1c.T@ea_T + (f_blk@Ws1b).T @ onehot_pe   (fj folded)
  msg  = (silu(h_s).T @ Ws2 + bs2) * fi    (lhsT-swap -> edge-on-partition)
  h_v / mv analogous; scatter rhs = [msg | mv*ev0 | mv*ev1 | mv*ev2].
"""

import sys

if "/opt/trn_rl_repo" not in sys.path:
    sys.path.insert(0, "/opt/trn_rl_repo")

from dataclasses import dataclass

import numpy as np

import concourse.bacc as bacc
import concourse.mybir as mybir
import concourse.tile as tile
from concourse.bass import IndirectOffsetOnAxis
from concourse.bass_utils import run_bass_kernel_spmd

F32 = mybir.dt.float32
F32R = mybir.dt.float32r
BF16 = mybir.dt.bfloat16
I32 = mybir.dt.int32
I16 = mybir.dt.int16

import ml_dtypes

BFNP = ml_dtypes.bfloat16
AF = mybir.ActivationFunctionType
OP = mybir.AluOpType

P = 128
D = 128  # node_dim == edge_dim == hidden
PAD_J = 300.0  # j_rel sentinel for padded edges (matches no partition)


@dataclass(frozen=True)
class Cfg:
    n_nodes: int = 25000
    n_edges: int = 400000
    n_species: int = 100
    e_in: int = 120
    n_cores: int = 8
    v_blocks: int = 26  # virtual node blocks per core
    mm_f32r: bool = False  # float32r needs producer-side rounding; keep off
    bf16: bool = True  # bf16 matmul inputs (fp32 PSUM accum, fp32 outputs)
    sigmoid_act: bool = False  # CoreSim lacks Silu; use Sigmoid for sim tests

    @property
    def npc(self):  # nodes per core
        return self.n_nodes // self.n_cores

    @property
    def vp(self):  # virtual node slots per core
        return self.v_blocks * P

    @property
    def n_pad(self):  # padded global node count (f table rows)
        return ((self.n_nodes + P - 1) // P) * P


FULL = Cfg()


# --------------------------------------------------------------------------
# Host-side prep: shard edges by destination range, bin-pack nodes to blocks
# --------------------------------------------------------------------------

def _prep(cfg: Cfg, inputs: dict):
    sp = np.asarray(inputs["species"], np.float32)
    ei = np.asarray(inputs["edge_index"])
    ea = np.asarray(inputs["edge_attr"], np.float32)
    ev = np.asarray(inputs["edge_vec"], np.float32)
    tt = np.asarray(inputs["t"], np.float32)
    i_g = ei[0].astype(np.int64)
    j_g = ei[1].astype(np.int64)
    V, NPC = cfg.v_blocks, cfg.npc

    cores = []
    b_req = 1
    for c in range(cfg.n_cores):
        lo = c * NPC
        eids = np.nonzero((j_g >= lo) & (j_g < lo + NPC))[0]
        jl = (j_g[eids] - lo).astype(np.int64)
        deg = np.bincount(jl, minlength=NPC)
        order = np.argsort(-deg, kind="stable")
        bin_e = np.zeros(V, np.int64)
        bin_n = np.zeros(V, np.int64)
        vslot = np.full(NPC, -1, np.int64)
        for n in order:
            elig = np.nonzero(bin_n < P)[0]
            b = elig[np.argmin(bin_e[elig])]
            vslot[n] = b * P + bin_n[b]
            bin_n[b] += 1
            bin_e[b] += deg[n]
        b_req = max(b_req, int(bin_e.max()))
        cores.append((eids, jl, vslot))

    B = max(512, ((b_req + 511) // 512) * 512)
    E_pad = V * B
    S_tot = E_pad // P

    per_core, perms = [], []
    for c, (eids, jl, vslot) in enumerate(cores):
        blk = vslot[jl] // P
        order_e = np.argsort(blk, kind="stable")
        e_s = eids[order_e]
        cnt = np.bincount(blk[order_e], minlength=V)
        perm = np.full(E_pad, -1, np.int64)
        off = 0
        for b in range(V):
            k = int(cnt[b])
            perm[b * B:b * B + k] = e_s[off:off + k]
            off += k
        valid = perm >= 0
        pc = np.where(valid, perm, 0)
        ii = np.where(valid, i_g[pc], 0).astype(np.int32)
        jl_pc = np.clip(j_g[pc] - c * NPC, 0, NPC - 1)
        jr = np.where(valid, vslot[jl_pc] % P, PAD_J).astype(np.float32)
        ea_p = np.where(valid[:, None], ea[pc], 0.0).astype(np.float32)
        ev_p = np.where(valid[:, None], ev[pc], 0.0).astype(np.float32)

        dtn = BFNP if cfg.bf16 else np.float32
        d = {}
        d["ea_t"] = np.ascontiguousarray(ea_p.T.astype(dtn))
        d["icol"] = np.ascontiguousarray(ii.reshape(S_tot, P).T)
        d["jcol"] = np.ascontiguousarray(jr.reshape(S_tot, P).T)
        d["jrow"] = np.ascontiguousarray(jr.reshape(V, B).astype(dtn))
        d["evcol"] = np.ascontiguousarray(np.concatenate(
            [ev_p[:, k].reshape(S_tot, P).T for k in range(3)], axis=1))

        perm_n = np.full(cfg.vp, -1, np.int64)
        perm_n[vslot] = np.arange(NPC) + c * NPC
        vn = perm_n >= 0
        pn = np.where(vn, perm_n, 0)
        spv = np.where(vn[None, :], sp[pn].T, 0.0).astype(np.float32)
        d["sptv"] = np.concatenate(
            [spv, np.ones((1, cfg.vp), np.float32)], 0).astype(dtn)
        d["trow1"] = np.ascontiguousarray(
            np.where(vn, tt[pn, 0], 0.0).reshape(1, cfg.vp).astype(np.float32))
        per_core.append(d)
        perms.append((perm, perm_n))

    # shared (replicated) arrays
    w = {k: np.asarray(inputs[k], np.float32) for k in (
        "Wa", "ba", "Wb1", "bb1", "Wb2", "bb2", "Ws1", "bs1", "Ws2", "bs2",
        "Wv1", "bv1", "Wv2", "bv2", "Wh1", "bh1", "Wh2", "bh2",
        "B_rff", "Wt1", "bt1", "Wt2", "bt2")}
    dtn = BFNP if cfg.bf16 else np.float32
    spt = np.zeros((cfg.n_species + 1, cfg.n_pad), np.float32)
    spt[:cfg.n_species, :cfg.n_nodes] = sp.T
    spt[cfg.n_species, :] = 1.0
    col = lambda v: np.ascontiguousarray(v.reshape(-1, 1))
    wt = lambda a: np.ascontiguousarray(a.astype(dtn))  # matmul weights
    shared = {
        "spt": spt.astype(dtn),
        "wa": wt(np.concatenate([w["Wa"], w["ba"][None, :]], 0)),
        "wb1": wt(w["Wb1"]), "bb1": col(w["bb1"]),
        "wb2": wt(w["Wb2"]), "bb2": col(w["bb2"]),
        "ws1a": wt(w["Ws1"][0:D]),
        "ws1b": wt(w["Ws1"][D:2 * D]),
        "ws1c": wt(w["Ws1"][2 * D:3 * D]),
        "bs1": col(w["bs1"]),
        "ws2": wt(w["Ws2"]), "bs2b": np.tile(w["bs2"][None, :], (P, 4)),
        "wv1a": wt(w["Wv1"][0:D]),
        "wv1b": wt(w["Wv1"][D:2 * D]),
        "wv1c": wt(w["Wv1"][2 * D:3 * D]),
        "bv1": col(w["bv1"]),
        "wv2": wt(w["Wv2"]), "bv2b": np.tile(w["bv2"][None, :], (P, 4)),
        "wh1f": wt(w["Wh1"][0:D]),
        "wh1a": wt(w["Wh1"][D:2 * D]),
        "bh1": col(w["bh1"]), "wh2": wt(w["Wh2"]),
        "wt1": wt(w["Wt1"]), "bt1": col(w["bt1"]), "wt2": wt(w["Wt2"]),
        "bh2t2": col(w["bh2"] + w["bt2"]),
        "brff2": col(np.concatenate([w["B_rff"][0], w["B_rff"][0]])),
        # time-embedding range reduction happens in "turns" space:
        # a = t*B + shift, with +8 bias so a>=0, +0.25 on the cos half.
        # A compare-subtract ladder brings a into [-0.5, 0.5).
        "shiftc": col(np.concatenate(
            [np.full(64, 8.25, np.float32), np.full(64, 8.0, np.float32)])),
        "zeroc": np.zeros((P, 1), np.float32),
        "iota_t": np.tile(np.arange(P, dtype=np.float32)[None, :],
                          (P, 1)).astype(dtn),
        "iota_c": np.arange(P, dtype=np.float32).reshape(P, 1),
        "ident": np.eye(P, dtype=np.float32),
        "ones_r": np.ones((1, P), np.float32),
        "ones_b": np.ones((1, P), dtn),
        "ident_b": np.eye(P, dtype=np.float32).astype(dtn),
    }
    assert np.abs(w["B_rff"]).max() < 7.5, "time RFF range ladder exceeded"
    return B, shared, per_core, perms


# --------------------------------------------------------------------------
# Device program (identical for every core)
# --------------------------------------------------------------------------

def build_program(cfg: Cfg, B: int):
    V = cfg.v_blocks
    E_pad = V * B
    S_tot = E_pad // P
    S = B // P        # sub-chunks (128 edges) per block
    CH = B // 512     # 512-edge chunks per block
    NG = cfg.n_pad // P
    EIN = cfg.e_in
    KSP = cfg.n_species + 1

    nc = bacc.Bacc("TRN2", target_bir_lowering=False, debug=False,
                   num_devices=cfg.n_cores)
    SILU = AF.Sigmoid if cfg.sigmoid_act else AF.Silu
    DT = BF16 if cfg.bf16 else F32

    def mm(ap):
        return ap.bitcast(F32R) if cfg.mm_f32r else ap

    din = {}
    specs = [
        ("spt", [KSP, cfg.n_pad], DT), ("sptv", [KSP, cfg.vp], DT),
        ("ea_t", [EIN, E_pad], DT), ("icol", [P, S_tot], I32),
        ("jcol", [P, S_tot], F32), ("jrow", [V, B], DT),
        ("evcol", [P, 3 * S_tot], F32), ("trow1", [1, cfg.vp], F32),
        ("wa", [KSP, D], DT),
        ("wb1", [EIN, D], DT), ("bb1", [D, 1], F32),
        ("wb2", [D, D], DT), ("bb2", [D, 1], F32),
        ("ws1a", [D, D], DT), ("ws1b", [D, D], DT), ("ws1c", [D, D], DT),
        ("bs1", [D, 1], F32), ("ws2", [D, D], DT), ("bs2b", [P, 512], F32),
        ("wv1a", [D, D], DT), ("wv1b", [D, D], DT), ("wv1c", [D, D], DT),
        ("bv1", [D, 1], F32), ("wv2", [D, D], DT), ("bv2b", [P, 512], F32),
        ("wh1f", [D, D], DT), ("wh1a", [D, D], DT), ("bh1", [D, 1], F32),
        ("wh2", [D, D], DT), ("wt1", [D, D], DT), ("bt1", [D, 1], F32),
        ("wt2", [D, D], DT), ("bh2t2", [D, 1], F32), ("brff2", [P, 1], F32),
        ("shiftc", [P, 1], F32), ("zeroc", [P, 1], F32),
        ("iota_t", [P, P], DT), ("iota_c", [P, 1], F32),
        ("ident", [P, P], F32), ("ones_r", [1, P], F32),
        ("ones_b", [1, P], DT), ("ident_b", [P, P], DT),
    ]
    for name, shape, dt in specs:
        din[name] = nc.dram_tensor(name, shape, dt, kind="ExternalInput").ap()

    ea_o = nc.dram_tensor("ea_o", [P, E_pad], F32, kind="ExternalOutput").ap()
    h0t_o = nc.dram_tensor("h0t", [P, cfg.vp], F32, kind="ExternalOutput").ap()
    v0b_o = nc.dram_tensor("v0b", [V, P, 512], F32, kind="ExternalOutput").ap()

    with tile.TileContext(nc) as tc:
        import contextlib
        ctx = contextlib.ExitStack()
        const = ctx.enter_context(tc.tile_pool(name="const", bufs=1))
        io = ctx.enter_context(tc.tile_pool(name="io", bufs=3))
        wk = ctx.enter_context(tc.tile_pool(name="wk", bufs=2))
        ps = ctx.enter_context(tc.tile_pool(name="ps", bufs=5, space="PSUM"))
        acc = ctx.enter_context(tc.tile_pool(name="acc", bufs=2, space="PSUM"))
        dpool = ctx.enter_context(tc.tile_pool(name="dsc", bufs=1, space="DRAM"))

        # constants into SBUF
        cs = {k: const.tile_from(din[k], name=f"c_{k}")
              for k in din if k not in ("spt", "sptv", "ea_t", "jrow")}
        f_tab = dpool.tile([cfg.n_pad, D], DT, name="f_tab")
        ftv = const.tile([P, cfg.vp], DT, name="ftv")

        # ---- Phase 1: node features f (rows to HBM, local f_T to SBUF) ----
        for g in range(NG):
            spg = io.tile([KSP, P], DT, name="spg", tag="spg")
            nc.sync.dma_start(out=spg[:], in_=din["spt"][:, g * P:(g + 1) * P])
            fps = ps.tile([P, 512], F32, name="fps", tag="ps")
            nc.tensor.matmul(fps[:, :P], lhsT=mm(spg[:]), rhs=mm(cs["wa"][:]),
                             start=True, stop=True)
            fsb = wk.tile([P, P], DT, name="fsb", tag="fsb")
            nc.scalar.copy(fsb[:], fps[:, :P])
            nc.sync.dma_start(out=f_tab[g * P:(g + 1) * P, :], in_=fsb[:])
        for b in range(V):
            spv = io.tile([KSP, P], DT, name="spv", tag="spg")
            nc.sync.dma_start(out=spv[:], in_=din["sptv"][:, b * P:(b + 1) * P])
            ftp = ps.tile([P, 512], F32, name="ftp", tag="ps")
            nc.tensor.matmul(ftp[:, :P], lhsT=mm(cs["wa"][:]), rhs=mm(spv[:]),
                             start=True, stop=True)
            nc.scalar.copy(ftv[:, b * P:(b + 1) * P], ftp[:, :P])

        # ---- Phase 1b: time embedding rff_T for all local nodes ----
        # turns a = t*B + shift(=8 / 8.25-for-cos); ladder-subtract to
        # [-0.5, 0.5); angle = 2*pi*a; rff = Sin(angle).
        rff_all = const.tile([P, cfg.vp], DT, name="rff_all")
        for g in range((cfg.vp + 511) // 512):
            n0 = g * 512
            w = min(cfg.vp, n0 + 512) - n0
            tbp = ps.tile([P, 512], F32, name="tbp", tag="ps")
            nc.tensor.matmul(tbp[:, :w], lhsT=mm(cs["ones_r"][:]),
                             rhs=mm(cs["trow1"][:, n0:n0 + w]),
                             start=True, stop=True)
            av = wk.tile([P, 512], F32, name="av", tag="av")
            nc.vector.tensor_scalar(av[:, :w], tbp[:, :w], cs["brff2"][:],
                                    cs["shiftc"][:], OP.mult, OP.add)
            for thr, amt in ((8.0, 8.0), (4.0, 4.0), (2.0, 2.0), (1.0, 1.0),
                             (0.5, 1.0)):
                gt = wk.tile([P, 512], F32, name="gt", tag="gt")
                nc.vector.tensor_scalar(gt[:, :w], av[:, :w], float(thr),
                                        float(amt), OP.is_ge, OP.mult)
                nc.vector.tensor_tensor(out=av[:, :w], in0=av[:, :w],
                                        in1=gt[:, :w], op=OP.subtract)
            ang = wk.tile([P, 512], F32, name="ang", tag="ang")
            nc.vector.tensor_scalar(ang[:, :w], av[:, :w],
                                    float(2.0 * np.pi), None, OP.mult)
            nc.scalar.activation(rff_all[:, n0:n0 + w], ang[:, :w], AF.Sin,
                                 bias=cs["zeroc"][:])

        # ---- Phase 2: edge pipeline + per-block scatter ----
        # The scatter matmuls of chunk (b, c) are emitted one chunk later
        # (after chunk (b, c+1)'s front half) so the PE never stalls in-order
        # behind DVE-produced msg/mv tiles - keeps HAM warm.
        def emit_scatter(pc_, ohep, rhs_list, accb):
            for q in range(4):
                s_in_b = pc_ * 4 + q
                for t in range(4):
                    nc.tensor.matmul(
                        accb[:, t * P:(t + 1) * P],
                        lhsT=mm(ohep[:, q * P:(q + 1) * P]),
                        rhs=mm(rhs_list[t][:, q * P:(q + 1) * P]),
                        start=(s_in_b == 0 and t == 0),
                        stop=(s_in_b == S - 1 and t == 3))

        def emit_epilogue(b, accb, ftb):
            # ---- block epilogue: store v0, node MLPs -> h0 ----
            accs = wk.tile([P, 512], F32, name="accs", tag="accs")
            nc.scalar.copy(accs[:], accb[:])
            nc.sync.dma_start(out=v0b_o[b, :, :], in_=accs[:])
            agp = ps.tile([P, 512], F32, name="agp", tag="ps")
            nc.tensor.matmul(agp[:, :P], lhsT=accs[:, :P], rhs=cs["ident"][:],
                             is_transpose=True, start=True, stop=True)
            agt = wk.tile([P, P], DT, name="agt", tag="agt")
            nc.scalar.copy(agt[:], agp[:, :P])
            hhp = ps.tile([P, 512], F32, name="hhp", tag="ps")
            nc.tensor.matmul(hhp[:, :P], lhsT=mm(cs["wh1f"][:]), rhs=mm(ftb),
                             start=True, stop=False)
            nc.tensor.matmul(hhp[:, :P], lhsT=mm(cs["wh1a"][:]), rhs=mm(agt[:]),
                             start=False, stop=True)
            hh = wk.tile([P, P], DT, name="hh", tag="hh")
            nc.scalar.activation(hh[:], hhp[:, :P], SILU, bias=cs["bh1"][:])
            # time embedding MLP (rff_T precomputed in phase 1b)
            htp = ps.tile([P, 512], F32, name="htp", tag="ps")
            nc.tensor.matmul(htp[:, :P], lhsT=mm(cs["wt1"][:]),
                             rhs=mm(rff_all[:, b * P:(b + 1) * P]),
                             start=True, stop=True)
            ht = wk.tile([P, P], DT, name="ht", tag="ht")
            nc.scalar.activation(ht[:], htp[:, :P], SILU, bias=cs["bt1"][:])
            h0p = ps.tile([P, 512], F32, name="h0p", tag="ps")
            nc.tensor.matmul(h0p[:, :P], lhsT=mm(cs["wh2"][:]), rhs=mm(hh[:]),
                             start=True, stop=False)
            nc.tensor.matmul(h0p[:, :P], lhsT=mm(cs["wt2"][:]), rhs=mm(ht[:]),
                             start=False, stop=True)
            h0s = wk.tile([P, P], F32, name="h0s", tag="h0s")
            nc.scalar.activation(h0s[:], h0p[:, :P], AF.Identity,
                                 bias=cs["bh2t2"][:])
            nc.sync.dma_start(out=h0t_o[:, b * P:(b + 1) * P], in_=h0s[:])

        pending = None
        for b in range(V):
            ftb = ftv[:, b * P:(b + 1) * P]
            gsp = ps.tile([P, 512], F32, name="gsp", tag="ps")
            nc.tensor.matmul(gsp[:, :P], lhsT=mm(ftb), rhs=mm(cs["ws1b"][:]),
                             start=True, stop=True)
            g_s = wk.tile([P, P], DT, name="g_s", tag="g_s")
            nc.scalar.copy(g_s[:], gsp[:, :P])
            gvp = ps.tile([P, 512], F32, name="gvp", tag="ps")
            nc.tensor.matmul(gvp[:, :P], lhsT=mm(ftb), rhs=mm(cs["wv1b"][:]),
                             start=True, stop=True)
            g_v = wk.tile([P, P], DT, name="g_v", tag="g_v")
            nc.scalar.copy(g_v[:], gvp[:, :P])

            accb = acc.tile([P, 512], F32, name="accb", tag="acc")
            for c in range(CH):
                e0 = b * B + c * 512
                s0 = e0 // P
                # loads
                eat = io.tile([EIN, 512], DT, name="eat", tag="eat")
                nc.sync.dma_start(out=eat[:], in_=din["ea_t"][:, e0:e0 + 512])
                jrw = io.tile([1, 512], DT, name="jrw", tag="jrw")
                nc.sync.dma_start(out=jrw[:],
                                  in_=din["jrow"][b:b + 1, c * 512:(c + 1) * 512])
                # j broadcast (K=1 matmul) -> onehot_pe
                jbp = ps.tile([P, 512], F32, name="jbp", tag="ps")
                nc.tensor.matmul(jbp[:], lhsT=mm(cs["ones_b"][:]), rhs=mm(jrw[:]),
                                 start=True, stop=True)
                ohpe = wk.tile([P, 512], DT, name="ohpe", tag="ohpe")
                nc.vector.tensor_scalar(ohpe[:], jbp[:], cs["iota_c"][:], None,
                                        OP.is_equal)
                # MLP_b -> ea (edge embedding, [d, e] layout)
                hbp = ps.tile([P, 512], F32, name="hbp", tag="ps")
                nc.tensor.matmul(hbp[:], lhsT=mm(cs["wb1"][:]), rhs=mm(eat[:]),
                                 start=True, stop=True)
                hb = wk.tile([P, 512], DT, name="hb", tag="hb")
                nc.scalar.activation(hb[:], hbp[:], SILU, bias=cs["bb1"][:])
                eap = ps.tile([P, 512], F32, name="eap", tag="ps")
                nc.tensor.matmul(eap[:], lhsT=mm(cs["wb2"][:]), rhs=mm(hb[:]),
                                 start=True, stop=True)
                eao = wk.tile([P, 512], F32, name="eao", tag="eao")
                nc.scalar.activation(eao[:], eap[:], AF.Identity, bias=cs["bb2"][:])
                nc.sync.dma_start(out=ea_o[:, e0:e0 + 512], in_=eao[:])
                if cfg.bf16:
                    eab = wk.tile([P, 512], DT, name="eab", tag="eab")
                    nc.scalar.copy(eab[:], eao[:])
                else:
                    eab = eao
                # fi gather (rows), and fi_T either via dma_gather (bf16)
                # or PE transposes (fp32)
                fic = wk.tile([P, 512], DT, name="fic", tag="fic")
                for q in range(4):
                    nc.gpsimd.indirect_dma_start(
                        out=fic[:, q * P:(q + 1) * P], out_offset=None,
                        in_=f_tab[:, :],
                        in_offset=IndirectOffsetOnAxis(
                            ap=cs["icol"][:, s0 + q:s0 + q + 1], axis=0))
                fit = wk.tile([P, 512], DT, name="fit", tag="fit")
                fitp = ps.tile([P, 512], DT, name="fitp", tag="ps")
                idn = cs["ident_b"] if cfg.bf16 else cs["ident"]
                for q in range(4):
                    nc.tensor.matmul(fitp[:, q * P:(q + 1) * P],
                                     lhsT=fic[:, q * P:(q + 1) * P],
                                     rhs=idn[:], is_transpose=True,
                                     start=(q == 0), stop=(q == 3))
                nc.scalar.copy(fit[:], fitp[:])
                # MLP_s hidden: Ws1a.T@fi_T + Ws1c.T@ea_T + G_s.T@onehot_pe
                hsp = ps.tile([P, 512], F32, name="hsp", tag="ps")
                nc.tensor.matmul(hsp[:], lhsT=mm(cs["ws1a"][:]), rhs=mm(fit[:]),
                                 start=True, stop=False)
                nc.tensor.matmul(hsp[:], lhsT=mm(cs["ws1c"][:]), rhs=mm(eab[:]),
                                 start=False, stop=False)
                nc.tensor.matmul(hsp[:], lhsT=mm(g_s[:]), rhs=mm(ohpe[:]),
                                 start=False, stop=True)
                hs = wk.tile([P, 512], DT, name="hs", tag="hs")
                nc.scalar.activation(hs[:], hsp[:], SILU, bias=cs["bs1"][:])
                # MLP_v hidden
                hvp = ps.tile([P, 512], F32, name="hvp", tag="ps")
                nc.tensor.matmul(hvp[:], lhsT=mm(cs["wv1a"][:]), rhs=mm(fit[:]),
                                 start=True, stop=False)
                nc.tensor.matmul(hvp[:], lhsT=mm(cs["wv1c"][:]), rhs=mm(eab[:]),
                                 start=False, stop=False)
                nc.tensor.matmul(hvp[:], lhsT=mm(g_v[:]), rhs=mm(ohpe[:]),
                                 start=False, stop=True)
                hv = wk.tile([P, 512], DT, name="hv", tag="hv")
                nc.scalar.activation(hv[:], hvp[:], SILU, bias=cs["bv1"][:])
                # second layers via lhsT-swap -> [e, d] layout
                msq = ps.tile([P, 512], F32, name="msq", tag="ps")
                for q in range(4):
                    nc.tensor.matmul(msq[:, q * P:(q + 1) * P],
                                     lhsT=mm(hs[:, q * P:(q + 1) * P]),
                                     rhs=mm(cs["ws2"][:]),
                                     start=(q == 0), stop=(q == 3))
                mvq = ps.tile([P, 512], F32, name="mvq", tag="ps")
                for q in range(4):
                    nc.tensor.matmul(mvq[:, q * P:(q + 1) * P],
                                     lhsT=mm(hv[:, q * P:(q + 1) * P]),
                                     rhs=mm(cs["wv2"][:]),
                                     start=(q == 0), stop=(q == 3))
                # onehot_ep  (lhsT of the scatter matmuls)
                ohep = wk.tile([P, 512], DT, name="ohep", tag="ohep")
                for q in range(4):
                    nc.vector.tensor_scalar(ohep[:, q * P:(q + 1) * P],
                                            cs["iota_t"][:],
                                            cs["jcol"][:, s0 + q:s0 + q + 1],
                                            None, OP.is_equal)
                # msg = (msq + bs2) * fi ;  mv_b = mvq + bv2 ; mv_k = mv_b*ev_k
                mtmp = wk.tile([P, 512], DT, name="mtmp", tag="mtmp")
                nc.vector.tensor_tensor(out=mtmp[:], in0=msq[:], in1=cs["bs2b"][:],
                                        op=OP.add)
                msg = wk.tile([P, 512], DT, name="msg", tag="msg")
                nc.vector.tensor_tensor(out=msg[:], in0=mtmp[:], in1=fic[:],
                                        op=OP.mult)
                mvb = wk.tile([P, 512], DT, name="mvb", tag="mvb")
                nc.vector.tensor_tensor(out=mvb[:], in0=mvq[:], in1=cs["bv2b"][:],
                                        op=OP.add)
                mvk = [wk.tile([P, 512], DT, name=f"mv{k}", tag=f"mv{k}")
                       for k in range(3)]
                for q in range(4):
                    for k in range(3):
                        nc.vector.tensor_scalar(
                            mvk[k][:, q * P:(q + 1) * P],
                            mvb[:, q * P:(q + 1) * P],
                            cs["evcol"][:, k * S_tot + s0 + q:k * S_tot + s0 + q + 1],
                            None, OP.mult)
                # scatter of the PREVIOUS chunk (1-chunk software pipeline)
                if pending is not None:
                    emit_scatter(*pending[1:])
                    if pending[0] is not None:
                        emit_epilogue(*pending[0])
                epi = (b, accb, ftb) if c == CH - 1 else None
                pending = (epi, c, ohep, [msg, mvk[0], mvk[1], mvk[2]], accb)

        if pending is not None:
            emit_scatter(*pending[1:])
            if pending[0] is not None:
                emit_epilogue(*pending[0])

        ctx.close()

    nc.compile()
    return nc


# --------------------------------------------------------------------------
# Runner
# --------------------------------------------------------------------------

_PROG_CACHE = {}


def _get_program(cfg: Cfg, B: int):
    key = (cfg, B)
    if key not in _PROG_CACHE:
        _PROG_CACHE[key] = build_program(cfg, B)
    return _PROG_CACHE[key]


def _unshard(cfg: Cfg, results, perms):
    N, E, V = cfg.n_nodes, cfg.n_edges, cfg.v_blocks
    h0 = np.zeros((N, D), np.float32)
    v0 = np.zeros((N, D, 3), np.float32)
    ea = np.zeros((E, D), np.float32)
    for res, (perm, perm_n) in zip(results, perms):
        vn = perm_n >= 0
        h0[perm_n[vn]] = res["h0t"].T[vn]
        vv = res["v0b"][:, :, P:].reshape(V * P, 3, D).transpose(0, 2, 1)
        v0[perm_n[vn]] = vv[vn]
        ve = perm >= 0
        ea[perm[ve]] = res["ea_o"].T[ve]
    return h0, v0, ea


def run(inputs: dict, cfg: Cfg = FULL, trace: bool = False):
    B, shared, per_core, perms = _prep(cfg, inputs)
    nc = _get_program(cfg, B)
    in_maps = [{**shared, **d} for d in per_core]
    res = run_bass_kernel_spmd(nc, in_maps, core_ids=list(range(cfg.n_cores)),
                               trace=trace)
    outs = _unshard(cfg, res.results, perms)
    return outs, res


def kernel(**inputs):
    (h0, v0, ea), _ = run(inputs)
    return h0, v0, ea
